# revision 1
# baseline (speedup 1.0000x reference)
"""Causal attention (B=4096, T=64, C=64) on 8 TRN2 NeuronCores, pure data parallel.

Per core: x shard [512, 64, 64]. 512-token macro-tiles (8 batches), bf16 matmul
operands (f32 PSUM accumulate), 2-way tile_position packing: even batches live on
partitions 0-63, odd batches on partitions 64-127, so per-batch matmuls run as
concurrent pairs in the two halves of the PE array.

Per tile:
  x loaded permuted (partition p <- tokens 4p..4p+3, 1KB contiguous descriptors);
  PE transposes write strided psum columns to restore natural token order.
  xT = transpose(x)                     [64, 512] bf16
  qT2/kT2[p, j, t]: rows 0:64 = even batches, 64:128 = odd  (strided rhs views)
  v[p, j, c] = xT_chunk.T @ WvT         (token-chunk layout == parity split)
  weiT psum = causal mask (identity matmul) + kT_b.T @ qT_b (accumulated);
  weiT_e = exp(0.125 * psum) -> bf16    (ACT reads PSUM directly)
  sums[p, j] = weiT_e.T @ ones ; recip = 1/sums   (matches y chunk layout)
  outT[c half, j, t] = v_b.T @ weiT_e
  y[p, j, c] = outT_chunk.T @ WpT ; y = y*recip + bp ; DMA out
"""

import numpy as np

import concourse.bass as bass
import concourse.mybir as mybir
import concourse.tile as tile
import concourse.masks as masks
from concourse import bacc

F32 = mybir.dt.float32
BF16 = mybir.dt.bfloat16

N_CORES = 8
B, T, C = 4096, 64, 64
B_LOC = B // N_CORES  # 512 batches per core

MASK_VAL = -1e9


def build_nc(b_loc=B_LOC, batches_per_tile=8, reps=1):
    """Build the single-core Bass graph (SPMD: same graph on all 8 cores)."""
    assert b_loc % batches_per_tile == 0
    n_tiles = b_loc // batches_per_tile
    TOK = batches_per_tile * T              # tokens per macro tile (512)
    NCH = TOK // 128                        # 128-token chunks per tile (4)

    nc = bacc.Bacc("TRN2", target_bir_lowering=False, debug=False)

    x_ext = nc.declare_dram_parameter("x", [b_loc, T, C], F32, isOutput=False)
    Wk_ext = nc.declare_dram_parameter("Wk", [C, C], F32, isOutput=False)
    Wq_ext = nc.declare_dram_parameter("Wq", [C, C], F32, isOutput=False)
    Wv_ext = nc.declare_dram_parameter("Wv", [C, C], F32, isOutput=False)
    Wp_ext = nc.declare_dram_parameter("Wp", [C, C], F32, isOutput=False)
    bp_ext = nc.declare_dram_parameter("bp", [C], F32, isOutput=False)
    out_ext = nc.declare_dram_parameter("out", [b_loc, T, C], F32, isOutput=True)

    x_flat = x_ext.ap().rearrange("b t c -> (b t) c")
    out_flat = out_ext.ap().rearrange("b t c -> (b t) c")

    # maskT[s, t] = 0 where s <= t else MASK_VAL*8 (exp scale 0.125 applied after)
    m1 = np.where(
        np.arange(T)[:, None] <= np.arange(T)[None, :], 0.0, MASK_VAL * 8.0
    ).astype(np.float32)
    maskT2_dram = nc.inline_tensor(np.vstack([m1, m1]), name="maskT2_const")
    ident_dram = nc.inline_tensor(np.eye(128, dtype=np.float32), name="ident_const")

    with tile.TileContext(nc) as tc:
        with (
            tc.tile_pool(name="const", bufs=1) as constp,
            tc.tile_pool(name="xin", bufs=4) as xin_pool,
            tc.tile_pool(name="work", bufs=8) as work_pool,
            tc.tile_pool(name="yout", bufs=4) as yout_pool,
            tc.tile_pool(name="ps", bufs=2, space="PSUM") as ps,
        ):
            # ---- one-time constants ----
            ident = constp.tile([128, 128], F32)
            nc.sync.dma_start(ident[:], ident_dram.ap())
            maskT2f = constp.tile([128, T], F32)
            nc.sync.dma_start(maskT2f[:], maskT2_dram.ap())
            ident_bf = constp.tile([128, 128], BF16)
            masks.make_identity(nc, ident_bf[:])
            maskT2 = constp.tile([128, T], BF16)
            nc.vector.tensor_copy(maskT2[:], maskT2f[:])
            ones128 = constp.tile([128, 1], BF16)
            nc.vector.memset(ones128[:], 1.0)

            # weights: natural DMA (contiguous), PE transpose, cast to bf16
            wnat = constp.tile([C, 4 * C], F32)
            for i, w_ext in enumerate((Wq_ext, Wk_ext, Wv_ext, Wp_ext)):
                nc.sync.dma_start(wnat[:, i * C : (i + 1) * C], w_ext.ap())
            wT_ps = ps.tile([C, 4 * C], F32, tag="A")
            for i in range(4):
                nc.tensor.transpose(
                    wT_ps[:, i * C : (i + 1) * C],
                    wnat[:, i * C : (i + 1) * C],
                    ident[0:C, 0:C],
                )
            A_ps = ps.tile([C, C], F32, tag="B")
            nc.tensor.matmul(
                A_ps[:], wnat[:, 0 * C : 1 * C], wnat[:, 1 * C : 2 * C]
            )
            A_bf = constp.tile([C, C], BF16)
            nc.vector.tensor_copy(A_bf[:], A_ps[:])
            WpTf = constp.tile([C, C], F32)
            nc.vector.tensor_copy(WpTf[:], wT_ps[:, 3 * C : 4 * C])
            B_ps = ps.tile([C, C], F32, tag="C")
            nc.tensor.matmul(B_ps[:], wnat[:, 2 * C : 3 * C], WpTf[:])
            B64 = constp.tile([C, C], BF16)
            nc.vector.tensor_copy(B64[:], B_ps[:])

            # bias broadcast to [128, C] via ones-matmul (K=1)
            bp_row = constp.tile([1, C], F32)
            nc.sync.dma_start(bp_row[:], bp_ext.ap().unsqueeze(0))
            ones_row128 = constp.tile([1, 128], F32)
            nc.vector.memset(ones_row128[:], 1.0)
            bias_ps = ps.tile([128, C], F32, tag="D")
            nc.tensor.matmul(bias_ps[:], ones_row128[:], bp_row[:])
            bias_bc = constp.tile([128, C], F32)
            nc.vector.tensor_copy(bias_bc[:], bias_ps[:])

            rep_ctx = tc.For_i(0, reps, 1) if reps > 1 else None
            if rep_ctx is not None:
                rep_ctx.__enter__()
            assert n_tiles % 2 == 0
            for st in range(n_tiles // 2):
              st0 = st * 2 * TOK
              # one 256KB x DMA per 2 compute tiles; partition p <- tokens 4p..4p+3
              x_sb2 = xin_pool.tile([128, 2, NCH * C], F32, tag="x_sb")
              for uu in range(2):
                  nc.sync.dma_start(
                      x_sb2[:, uu, :],
                      x_flat[st0 + uu * TOK : st0 + (uu + 1) * TOK, :].rearrange(
                          "(p m) c -> p (m c)", m=4
                      ),
                  )
              y_sb2 = yout_pool.tile([128, 2, NCH, C], F32, tag="y_sb")
              for u in range(2):
                  t0 = st0 + u * TOK
                  x_sb = x_sb2[:, u, :]

                  # ---- transpose (permuted cols), un-permute in the copy ----
                  xT_ps = ps.tile([C, TOK], F32, tag="A")
                  for m in range(NCH):
                      nc.tensor.transpose(
                          xT_ps[:, m * 128 : (m + 1) * 128],
                          x_sb[:, m * C : (m + 1) * C],
                          ident[:],
                      )
                  xT = work_pool.tile([C, TOK], BF16, tag="xT")
                  nc.scalar.copy(
                      xT[:].rearrange("c (p m) -> c m p", m=4),
                      xT_ps[:].rearrange("c (m p) -> c m p", p=128),
                  )
                  # ---- hT = (x @ A)T with A = Wq.T @ Wk ; scores = xT.T @ hT ----
                  hT_ps = ps.tile([C, TOK], F32, tag="B")
                  nc.tensor.matmul(hT_ps[:], A_bf[:], xT[:])
                  hT = work_pool.tile([C, TOK], BF16, tag="hT")
                  nc.scalar.copy(hT[:], hT_ps[:])

                  # ---- scores: one mask fill (K=128, bcast rhs), += kT.T@qT ----
                  weiT_ps = ps.tile([128, NCH, T], F32, tag="D")
                  nc.tensor.matmul(
                      weiT_ps[:],
                      ident_bf[:],
                      maskT2[:].unsqueeze(1).broadcast_to([128, NCH, T]),
                      start=True, stop=False, skip_group_check=True,
                  )
                  for j in range(NCH):
                      be, bo = 2 * j, 2 * j + 1
                      nc.tensor.matmul(
                          weiT_ps[0:T, j, :],
                          xT[:, be * T : (be + 1) * T],
                          hT[:, be * T : (be + 1) * T],
                          start=False, stop=(j == NCH - 1),
                          skip_group_check=True,
                      )
                      nc.tensor.matmul(
                          weiT_ps[T:128, j, :],
                          xT[:, bo * T : (bo + 1) * T],
                          hT[:, bo * T : (bo + 1) * T],
                          start=False, stop=(j == NCH - 1),
                          tile_position=(0, 64), skip_group_check=True,
                      )
                  weiT_e = work_pool.tile([128, NCH, T], BF16, tag="weiT_e")
                  nc.scalar.activation(
                      weiT_e[:], weiT_ps[:], mybir.ActivationFunctionType.Exp,
                      scale=0.125,
                  )


                  # ---- xB = x @ (Wv.T Wp.T), natural chunks (off-chain) ----
                  xB_ps = ps.tile([128, NCH, C], F32, tag="A")
                  for j in range(NCH):
                      nc.tensor.matmul(
                          xB_ps[:, j, :], xT[:, j * 128 : (j + 1) * 128], B64[:]
                      )
                  xB = work_pool.tile([128, NCH, C + 1], BF16, tag="xB")
                  nc.vector.tensor_copy(xB[:, :, 0:C], xB_ps[:])
                  nc.vector.memset(xB[:, :, C : C + 1], 1.0)

                  # ---- y|sums = wei_e @ [xB|1], packed pairs (N=65) ----
                  y_ps = ps.tile([128, NCH, C + 2], F32, tag="C")
                  for j in range(NCH):
                      nc.tensor.matmul(
                          y_ps[0:T, j, 0 : C + 1],
                          weiT_e[0:T, j, :], xB[0:T, j, :],
                      )
                      nc.tensor.matmul(
                          y_ps[T:128, j, 0 : C + 1],
                          weiT_e[T:128, j, :],
                          xB[T:128, j, :],
                          tile_position=(64, 64),
                      )
                  recip = work_pool.tile([128, NCH], F32, tag="recip")
                  nc.vector.reciprocal(recip[:], y_ps[:, :, C : C + 1])
                  # ---- y = y*recip + bias ; store ----
                  y_sb = y_sb2[:, u, :, :]
                  for j in range(NCH):
                      nc.vector.scalar_tensor_tensor(
                          y_sb[:, j, :],
                          y_ps[:, j, 0:C],
                          recip[:, j : j + 1],
                          bias_bc[:],
                          mybir.AluOpType.mult,
                          mybir.AluOpType.add,
                      )

              # per-u 128KB y DMAs on gpsimd (SWDGE) - ship u=0 while u=1 computes
              for uu in range(2):
                  nc.gpsimd.dma_start(
                      out_flat[st0 + uu * TOK : st0 + (uu + 1) * TOK, :].rearrange(
                          "(j p) c -> p j c", p=128
                      ),
                      y_sb2[:, uu, :, :],
                  )
            if rep_ctx is not None:
                rep_ctx.__exit__(None, None, None)

    nc.compile()
    return nc


_NC_CACHE = {}


def _get_nc(b_loc, batches_per_tile=8):
    key = (b_loc, batches_per_tile)
    if key not in _NC_CACHE:
        _NC_CACHE[key] = build_nc(b_loc, batches_per_tile)
    return _NC_CACHE[key]


def kernel(x, Wk, Wq, Wv, Wp, bp):
    from concourse.bass_utils import run_bass_kernel_spmd

    x = np.ascontiguousarray(x, dtype=np.float32)
    weights = {
        "Wk": np.ascontiguousarray(Wk, dtype=np.float32),
        "Wq": np.ascontiguousarray(Wq, dtype=np.float32),
        "Wv": np.ascontiguousarray(Wv, dtype=np.float32),
        "Wp": np.ascontiguousarray(Wp, dtype=np.float32),
        "bp": np.ascontiguousarray(bp, dtype=np.float32),
    }
    nc = _get_nc(B_LOC)
    in_maps = [
        {"x": x[i * B_LOC : (i + 1) * B_LOC], **weights} for i in range(N_CORES)
    ]
    res = run_bass_kernel_spmd(nc, in_maps, core_ids=list(range(N_CORES)))
    outs = [res.results[i]["out"] for i in range(N_CORES)]
    return np.concatenate(outs, axis=0)



# revision 28
# speedup vs baseline: 1.3612x; 1.3612x over previous
"""Causal attention (B=4096, T=64, C=64) on 8 TRN2 NeuronCores, pure data parallel.

Per core: x shard [512, 64, 64]. 512-token tiles (8 batches), bf16 matmuls with
fused weights A=Wq^T Wk, B=Wv^T Wp^T, bias folded into xB.

Layout: x loaded permuted (partition p <- tokens 4p..4p+3, 1KB descriptors).
8 PE transposes per tile (K=64 halves at tile positions (0,0)/(64,64)) write
strided PSUM columns -> xT [128 parts = 2 token-halves x 64c, 256 tok] in
natural token order. hT = A^T@xT, scores = xT_b^T@hT_b (+causal mask matmul),
exp on ACT, xB = xT_b^T@B + bp, y split into even/odd-token matmuls so each
PSUM partition holds 2 consecutive tokens -> 512B output descriptors.
Output DMA on the ACT HWDGE queue; input DMA on SP; 4-tile DMA chunks.
Elementwise: ACT(hT copy, exp), DVE(xB+bias, recip, yscale lo), Pool(xT copy,
yscale hi).
"""

import numpy as np

import concourse.bass as bass
import concourse.mybir as mybir
import concourse.tile as tile
import concourse.masks as masks
from concourse import bacc

F32 = mybir.dt.float32
F32R = mybir.dt.float32r
BF16 = mybir.dt.bfloat16
ADD = mybir.AluOpType.add
MULT = mybir.AluOpType.mult

N_CORES = 8
B, T, C = 4096, 64, 64
B_LOC = B // N_CORES  # 512 batches per core

MASK_VAL = -1e9


def build_nc(b_loc=B_LOC, batches_per_tile=8, reps=1):
    """Build the single-core Bass graph (SPMD: same graph on all 8 cores)."""
    assert batches_per_tile == 8
    TOK = batches_per_tile * T          # tokens per tile (512)
    n_tiles = b_loc // batches_per_tile  # 64
    TPC = 2                              # tiles per DMA chunk
    n_chunks = n_tiles // TPC            # 16
    HTOK = TOK // 2                      # 256 tokens per half

    nc = bacc.Bacc("TRN2", target_bir_lowering=False, debug=False)

    x_ext = nc.declare_dram_parameter("x", [b_loc, T, C], F32, isOutput=False)
    Wk_ext = nc.declare_dram_parameter("Wk", [C, C], F32, isOutput=False)
    Wq_ext = nc.declare_dram_parameter("Wq", [C, C], F32, isOutput=False)
    Wv_ext = nc.declare_dram_parameter("Wv", [C, C], F32, isOutput=False)
    Wp_ext = nc.declare_dram_parameter("Wp", [C, C], F32, isOutput=False)
    bp_ext = nc.declare_dram_parameter("bp", [C], F32, isOutput=False)
    out_ext = nc.declare_dram_parameter("out", [b_loc, T, C], F32, isOutput=True)

    x_flat = x_ext.ap().rearrange("b t c -> (b t) c")
    out_flat = out_ext.ap().rearrange("b t c -> (b t) c")

    # maskT[s, t] = 0 where s <= t else MASK_VAL*8 (exp scale 0.125 applied after)
    m1 = np.where(
        np.arange(T)[:, None] <= np.arange(T)[None, :], 0.0, MASK_VAL * 8.0
    ).astype(np.float32)
    maskT2_dram = nc.inline_tensor(np.vstack([m1, m1]), name="maskT2_const")
    i64 = np.eye(64, dtype=np.float32)
    ident2_dram = nc.inline_tensor(np.vstack([i64, i64]), name="ident2_const")

    with tile.TileContext(nc) as tc:
        with (
            tc.tile_pool(name="const", bufs=1) as constp,
            tc.tile_pool(name="xin", bufs=2) as xin_pool,
            tc.tile_pool(name="work", bufs=3) as work_pool,
            tc.tile_pool(name="yout", bufs=2) as yout_pool,
            tc.tile_pool(name="ps", bufs=2, space="PSUM") as ps,
        ):
            # ---- one-time constants ----
            ident2 = constp.tile([128, 64], F32)
            nc.sync.dma_start(ident2[:], ident2_dram.ap())
            ident_bf = constp.tile([128, 128], BF16)
            masks.make_identity(nc, ident_bf[:])
            maskT2f = constp.tile([128, T], F32)
            nc.sync.dma_start(maskT2f[:], maskT2_dram.ap())
            maskT2 = constp.tile([128, T], BF16)
            nc.vector.tensor_copy(maskT2[:], maskT2f[:])

            # weights: natural DMA (contiguous), fused A = Wq^T Wk, B = Wv^T Wp^T
            wnat = constp.tile([C, 4 * C], F32)
            for i, w_ext in enumerate((Wq_ext, Wk_ext, Wv_ext, Wp_ext)):
                nc.sync.dma_start(wnat[:, i * C : (i + 1) * C], w_ext.ap())
            # Wp^T via PE transpose (f32)
            wT_ps = ps.tile([C, C], F32, tag="y")
            nc.tensor.transpose(
                wT_ps[:], wnat[:, 3 * C : 4 * C], ident2[0:C, 0:C]
            )
            WpTf = constp.tile([C, C], F32)
            nc.vector.tensor_copy(WpTf[:], wT_ps[:])
            # A and B, each replicated on both partition halves
            AB_ps = ps.tile([128, 2, C], F32, tag="xh")
            for h in (0, 1):
                nc.tensor.matmul(
                    AB_ps[h * C : (h + 1) * C, 0, :],
                    wnat[:, 0:C], wnat[:, C : 2 * C],
                    tile_position=(0, h * C),
                )
                nc.tensor.matmul(
                    AB_ps[h * C : (h + 1) * C, 1, :],
                    wnat[:, 2 * C : 3 * C], WpTf[:],
                    tile_position=(0, h * C),
                )
            A2 = constp.tile([128, C], BF16)
            nc.vector.tensor_copy(A2[:], AB_ps[:, 0, :])
            B2 = constp.tile([128, C], BF16)
            nc.vector.tensor_copy(B2[:], AB_ps[:, 1, :])

            # bias broadcast to [128, C] via ones-matmul (K=1)
            bp_row = constp.tile([1, C], F32)
            nc.sync.dma_start(bp_row[:], bp_ext.ap().unsqueeze(0))
            ones_row128 = constp.tile([1, 128], F32)
            nc.vector.memset(ones_row128[:], 1.0)
            bias_ps = ps.tile([128, C], F32, tag="wb")
            nc.tensor.matmul(bias_ps[:], ones_row128[:], bp_row[:])
            bias_bc = constp.tile([128, C], F32)
            nc.vector.tensor_copy(bias_bc[:], bias_ps[:])



            rep_ctx = tc.For_i(0, reps, 1) if reps > 1 else None
            if rep_ctx is not None:
                rep_ctx.__enter__()
            for st in range(n_chunks):
                base = st * TPC * TOK
                # one 512KB x DMA per 4 tiles; partition p <- tokens 4p..4p+3
                x4 = xin_pool.tile([128, TPC, 4 * C], F32, tag="x4")
                nc.sync.dma_start(
                    x4[:],
                    x_flat[base : base + TPC * TOK, :].rearrange(
                        "(u p m) c -> p u (m c)", u=TPC, p=128, m=4
                    ),
                )
                y4 = yout_pool.tile([128, TPC, 2, 2, C], F32, tag="y4")
                for u in range(TPC):
                    # xh bank: [:,0]=xT (viewed [64,4] strided), [:,1]=hT
                    xh_ps = ps.tile([128, 2, HTOK // 4, 4], F32, tag="xh")
                    # ---- transpose: 8x [64,64] regular f32r matmuls,
                    #      strided psum cols restore natural token order ----
                    for h in (0, 1):
                        for m in range(4):
                            nc.tensor.matmul(
                                xh_ps[h * C : (h + 1) * C, 0, :, m],
                                x4[h * C : (h + 1) * C, u, m * C : (m + 1) * C],
                                ident2[h * C : (h + 1) * C, :],
                                tile_position=(h * C, h * C),
                            )
                    xT = work_pool.tile([128, HTOK], BF16, tag="xT")
                    # split the psum->sbuf cast copy across ACT and DVE
                    nc.scalar.copy(
                        xT[:, 0 : HTOK // 2],
                        xh_ps[:, 0, 0 : HTOK // 8, :].rearrange(
                            "p a m -> p (a m)"
                        ),
                    )
                    nc.vector.tensor_copy(
                        xT[:, HTOK // 2 :],
                        xh_ps[:, 0, HTOK // 8 :, :].rearrange(
                            "p a m -> p (a m)"
                        ),
                    )
                    # ---- hT = A^T @ xT per half ----
                    for h in (0, 1):
                        nc.tensor.matmul(
                            xh_ps[h * C : (h + 1) * C, 1]
                            .rearrange("p a m -> p (a m)"),
                            A2[h * C : (h + 1) * C, :],
                            xT[h * C : (h + 1) * C, :],
                            tile_position=(h * C, h * C),
                        )
                    hT = work_pool.tile([128, HTOK], BF16, tag="hT")
                    nc.scalar.copy(
                        hT[:], xh_ps[:, 1].rearrange("p a m -> p (a m)")
                    )

                    # wb bank: [:,0]=wei, [:,1]=xB
                    wb_ps = ps.tile([128, 2, 4, T], F32, tag="wb")
                    nc.tensor.matmul(
                        wb_ps[:, 0],
                        ident_bf[:],
                        maskT2[:].unsqueeze(1).broadcast_to([128, 4, T]),
                        start=True, stop=False, skip_group_check=True,
                    )
                    for h in (0, 1):
                        for bl in range(4):
                            nc.tensor.matmul(
                                wb_ps[h * C : (h + 1) * C, 0, bl, :],
                                xT[h * C : (h + 1) * C, bl * T : (bl + 1) * T],
                                hT[h * C : (h + 1) * C, bl * T : (bl + 1) * T],
                                start=False, stop=(h == 1 and bl == 3),
                                tile_position=(h * C, h * C),
                                skip_group_check=True,
                            )
                    weiT_e = work_pool.tile([128, 4, T], BF16, tag="weiT_e")
                    nc.scalar.activation(
                        weiT_e[:], wb_ps[:, 0],
                        mybir.ActivationFunctionType.Exp,
                        scale=0.125,
                    )

                    # ---- xB = x @ B per batch ----
                    for h in (0, 1):
                        for bl in range(4):
                            nc.tensor.matmul(
                                wb_ps[h * C : (h + 1) * C, 1, bl, :],
                                xT[h * C : (h + 1) * C, bl * T : (bl + 1) * T],
                                B2[h * C : (h + 1) * C, :],
                                tile_position=(h * C, h * C),
                            )
                    xB = work_pool.tile([128, 4, C + 1], BF16, tag="xB")
                    nc.vector.tensor_tensor(
                        xB[:, :, 0:C],
                        wb_ps[:, 1],
                        bias_bc[:].unsqueeze(1).broadcast_to([128, 4, C]),
                        ADD,
                    )
                    nc.vector.memset(xB[:, :, C : C + 1], 1.0)

                    # ---- y | sums: even/odd token split -> 2 tok/partition ----
                    # two full psum banks: h selects the bank (max 8 matmul
                    # groups per bank); slots 2..3 of dim 'a' are padding
                    y_ps = ps.tile([128, 2, 4, 2 * C], F32, tag="y")
                    for h in (0, 1):
                        for bl in range(4):
                            for par in (0, 1):
                                nc.tensor.matmul(
                                    y_ps[bl * 32 : (bl + 1) * 32, h, par, 0 : C + 1],
                                    weiT_e[h * C : (h + 1) * C, bl, par::2],
                                    xB[h * C : (h + 1) * C, bl, :],
                                    tile_position=(h * C, bl * 32),
                                )
                    recip = work_pool.tile([128, 2, 2], F32, tag="recip")
                    nc.vector.reciprocal(recip[:], y_ps[:, :, 0:2, C : C + 1])
                    nc.vector.tensor_tensor(
                        y4[:, u, :, :, :],
                        y_ps[:, :, 0:2, 0:C],
                        recip[:].unsqueeze(3).broadcast_to([128, 2, 2, C]),
                        MULT,
                    )
                # out DMA on ACT hwdge queue; 512B descriptors
                nc.sync.dma_start(
                    out_flat[base : base + TPC * TOK, :].rearrange(
                        "(u j p m) c -> p u j (m c)", u=TPC, j=2, p=128, m=2
                    ),
                    y4[:].rearrange("p u j m c -> p u j (m c)"),
                )
            if rep_ctx is not None:
                rep_ctx.__exit__(None, None, None)

    nc.compile()
    return nc


_NC_CACHE = {}


def _get_nc(b_loc, batches_per_tile=8):
    key = (b_loc, batches_per_tile)
    if key not in _NC_CACHE:
        _NC_CACHE[key] = build_nc(b_loc, batches_per_tile)
    return _NC_CACHE[key]


def kernel(x, Wk, Wq, Wv, Wp, bp):
    from concourse.bass_utils import run_bass_kernel_spmd

    x = np.ascontiguousarray(x, dtype=np.float32)
    weights = {
        "Wk": np.ascontiguousarray(Wk, dtype=np.float32),
        "Wq": np.ascontiguousarray(Wq, dtype=np.float32),
        "Wv": np.ascontiguousarray(Wv, dtype=np.float32),
        "Wp": np.ascontiguousarray(Wp, dtype=np.float32),
        "bp": np.ascontiguousarray(bp, dtype=np.float32),
    }
    nc = _get_nc(B_LOC)
    in_maps = [
        {"x": x[i * B_LOC : (i + 1) * B_LOC], **weights} for i in range(N_CORES)
    ]
    res = run_bass_kernel_spmd(nc, in_maps, core_ids=list(range(N_CORES)))
    outs = [res.results[i]["out"] for i in range(N_CORES)]
    return np.concatenate(outs, axis=0)


# revision 30
# speedup vs baseline: 1.7457x; 1.2824x over previous
"""Causal attention (B=4096, T=64, C=64) on 8 TRN2 NeuronCores, pure data parallel.

Per core: x shard [512, 64, 64]. 512-token tiles (8 batches), bf16 matmuls with
fused weights A=Wq^T Wk, B=Wv^T Wp^T, bias folded into xB.

Layout: x loaded permuted (partition p <- tokens 4p..4p+3, 1KB descriptors).
8 PE transposes per tile (K=64 halves at tile positions (0,0)/(64,64)) write
strided PSUM columns -> xT [128 parts = 2 token-halves x 64c, 256 tok] in
natural token order. hT = A^T@xT, scores = xT_b^T@hT_b (+causal mask matmul),
exp on ACT, xB = xT_b^T@B + bp, y split into even/odd-token matmuls so each
PSUM partition holds 2 consecutive tokens -> 512B output descriptors.
Output DMA on the ACT HWDGE queue; input DMA on SP; 4-tile DMA chunks.
Elementwise: ACT(hT copy, exp), DVE(xB+bias, recip, yscale lo), Pool(xT copy,
yscale hi).
"""

import numpy as np

import concourse.bass as bass
import concourse.mybir as mybir
import concourse.tile as tile
import concourse.masks as masks
from concourse import bacc

F32 = mybir.dt.float32
F32R = mybir.dt.float32r
BF16 = mybir.dt.bfloat16
ADD = mybir.AluOpType.add
MULT = mybir.AluOpType.mult

N_CORES = 8
B, T, C = 4096, 64, 64
B_LOC = B // N_CORES  # 512 batches per core

MASK_VAL = -1e9


def build_nc(b_loc=B_LOC, batches_per_tile=8, reps=1):
    """Build the single-core Bass graph (SPMD: same graph on all 8 cores)."""
    assert batches_per_tile == 8
    TOK = batches_per_tile * T          # tokens per tile (512)
    n_tiles = b_loc // batches_per_tile  # 64
    TPC = 2                              # tiles per DMA chunk
    n_chunks = n_tiles // TPC            # 16
    HTOK = TOK // 2                      # 256 tokens per half

    nc = bacc.Bacc("TRN2", target_bir_lowering=False, debug=False)

    x_ext = nc.declare_dram_parameter("x", [b_loc, T, C], F32, isOutput=False)
    Wk_ext = nc.declare_dram_parameter("Wk", [C, C], F32, isOutput=False)
    Wq_ext = nc.declare_dram_parameter("Wq", [C, C], F32, isOutput=False)
    Wv_ext = nc.declare_dram_parameter("Wv", [C, C], F32, isOutput=False)
    Wp_ext = nc.declare_dram_parameter("Wp", [C, C], F32, isOutput=False)
    bp_ext = nc.declare_dram_parameter("bp", [C], F32, isOutput=False)
    out_ext = nc.declare_dram_parameter("out", [b_loc, T, C], F32, isOutput=True)

    x_flat = x_ext.ap().rearrange("b t c -> (b t) c")
    out_flat = out_ext.ap().rearrange("b t c -> (b t) c")

    # maskT[s, t] = 0 where s <= t else MASK_VAL*8 (exp scale 0.125 applied after)
    m1 = np.where(
        np.arange(T)[:, None] <= np.arange(T)[None, :], 0.0, MASK_VAL * 8.0
    ).astype(np.float32)
    maskT2_dram = nc.inline_tensor(np.vstack([m1, m1]), name="maskT2_const")
    i64 = np.eye(64, dtype=np.float32)
    ident2_dram = nc.inline_tensor(np.vstack([i64, i64]), name="ident2_const")

    with tile.TileContext(nc) as tc:
        with (
            tc.tile_pool(name="const", bufs=1) as constp,
            tc.tile_pool(name="xin", bufs=2) as xin_pool,
            tc.tile_pool(name="work", bufs=3) as work_pool,
            tc.tile_pool(name="yout", bufs=2) as yout_pool,
            tc.tile_pool(name="ps", bufs=2, space="PSUM") as ps,
        ):
            # ---- one-time constants ----
            ident2 = constp.tile([128, 64], F32)
            nc.sync.dma_start(ident2[:], ident2_dram.ap())
            ident2_bf = constp.tile([128, 64], BF16)
            nc.vector.tensor_copy(ident2_bf[:], ident2[:])
            ident_bf = constp.tile([128, 128], BF16)
            masks.make_identity(nc, ident_bf[:])
            maskT2f = constp.tile([128, T], F32)
            nc.sync.dma_start(maskT2f[:], maskT2_dram.ap())
            maskT2 = constp.tile([128, T], BF16)
            nc.vector.tensor_copy(maskT2[:], maskT2f[:])

            # weights: natural DMA (contiguous), fused A = Wq^T Wk, B = Wv^T Wp^T
            wnat = constp.tile([C, 4 * C], F32)
            for i, w_ext in enumerate((Wq_ext, Wk_ext, Wv_ext, Wp_ext)):
                nc.sync.dma_start(wnat[:, i * C : (i + 1) * C], w_ext.ap())
            # Wp^T via PE transpose (f32)
            wT_ps = ps.tile([C, C], F32, tag="y")
            nc.tensor.transpose(
                wT_ps[:], wnat[:, 3 * C : 4 * C], ident2[0:C, 0:C]
            )
            WpTf = constp.tile([C, C], F32)
            nc.vector.tensor_copy(WpTf[:], wT_ps[:])
            # A and B, each replicated on both partition halves
            AB_ps = ps.tile([128, 2, C], F32, tag="xh")
            for h in (0, 1):
                nc.tensor.matmul(
                    AB_ps[h * C : (h + 1) * C, 0, :],
                    wnat[:, 0:C], wnat[:, C : 2 * C],
                    tile_position=(0, h * C),
                )
                nc.tensor.matmul(
                    AB_ps[h * C : (h + 1) * C, 1, :],
                    wnat[:, 2 * C : 3 * C], WpTf[:],
                    tile_position=(0, h * C),
                )
            A2 = constp.tile([128, C], BF16)
            nc.vector.tensor_copy(A2[:], AB_ps[:, 0, :])
            B2 = constp.tile([128, C], BF16)
            nc.vector.tensor_copy(B2[:], AB_ps[:, 1, :])

            # bias broadcast to [128, C] via ones-matmul (K=1)
            bp_row = constp.tile([1, C], F32)
            nc.sync.dma_start(bp_row[:], bp_ext.ap().unsqueeze(0))
            ones_row128 = constp.tile([1, 128], F32)
            nc.vector.memset(ones_row128[:], 1.0)
            bias_ps = ps.tile([128, C], F32, tag="wb")
            nc.tensor.matmul(bias_ps[:], ones_row128[:], bp_row[:])
            bias_bc = constp.tile([128, C], F32)
            nc.vector.tensor_copy(bias_bc[:], bias_ps[:])



            rep_ctx = tc.For_i(0, reps, 1) if reps > 1 else None
            if rep_ctx is not None:
                rep_ctx.__enter__()
            for st in range(n_chunks):
                base = st * TPC * TOK
                # one 512KB x DMA per 4 tiles; partition p <- tokens 4p..4p+3
                x4 = xin_pool.tile([128, TPC, 4 * C], F32, tag="x4")
                nc.sync.dma_start(
                    x4[:],
                    x_flat[base : base + TPC * TOK, :].rearrange(
                        "(u p m) c -> p u (m c)", u=TPC, p=128, m=4
                    ),
                )
                y4 = yout_pool.tile([128, TPC, 2, 2, C], F32, tag="y4")
                for u in range(TPC):
                    # ---- cast x tile to bf16 on the (otherwise idle) Pool ----
                    x_bf = work_pool.tile([128, 4 * C], BF16, tag="x_bf")
                    nc.gpsimd.tensor_copy(x_bf[:], x4[:, u, :])
                    # xh bank: [:,0]=xT (viewed [64,4] strided), [:,1]=hT
                    xh_ps = ps.tile([128, 2, HTOK // 4, 4], F32, tag="xh")
                    # ---- transpose: 8x [64,64] regular bf16 matmuls,
                    #      strided psum cols restore natural token order ----
                    for h in (0, 1):
                        for m in range(4):
                            nc.tensor.matmul(
                                xh_ps[h * C : (h + 1) * C, 0, :, m],
                                x_bf[h * C : (h + 1) * C, m * C : (m + 1) * C],
                                ident2_bf[h * C : (h + 1) * C, :],
                                tile_position=(h * C, h * C),
                            )
                    xT = work_pool.tile([128, HTOK], BF16, tag="xT")
                    # split the psum->sbuf cast copy across ACT and DVE
                    nc.scalar.copy(
                        xT[:, 0 : HTOK // 2],
                        xh_ps[:, 0, 0 : HTOK // 8, :].rearrange(
                            "p a m -> p (a m)"
                        ),
                    )
                    nc.vector.tensor_copy(
                        xT[:, HTOK // 2 :],
                        xh_ps[:, 0, HTOK // 8 :, :].rearrange(
                            "p a m -> p (a m)"
                        ),
                    )
                    # ---- hT = A^T @ xT per half ----
                    for h in (0, 1):
                        nc.tensor.matmul(
                            xh_ps[h * C : (h + 1) * C, 1]
                            .rearrange("p a m -> p (a m)"),
                            A2[h * C : (h + 1) * C, :],
                            xT[h * C : (h + 1) * C, :],
                            tile_position=(h * C, h * C),
                        )
                    hT = work_pool.tile([128, HTOK], BF16, tag="hT")
                    nc.scalar.copy(
                        hT[:], xh_ps[:, 1].rearrange("p a m -> p (a m)")
                    )

                    # wb bank: [:,0]=wei, [:,1]=xB
                    wb_ps = ps.tile([128, 2, 4, T], F32, tag="wb")
                    nc.tensor.matmul(
                        wb_ps[:, 0],
                        ident_bf[:],
                        maskT2[:].unsqueeze(1).broadcast_to([128, 4, T]),
                        start=True, stop=False, skip_group_check=True,
                    )
                    for h in (0, 1):
                        for bl in range(4):
                            nc.tensor.matmul(
                                wb_ps[h * C : (h + 1) * C, 0, bl, :],
                                xT[h * C : (h + 1) * C, bl * T : (bl + 1) * T],
                                hT[h * C : (h + 1) * C, bl * T : (bl + 1) * T],
                                start=False, stop=(h == 1 and bl == 3),
                                tile_position=(h * C, h * C),
                                skip_group_check=True,
                            )
                    weiT_e = work_pool.tile([128, 4, T], BF16, tag="weiT_e")
                    nc.scalar.activation(
                        weiT_e[:], wb_ps[:, 0],
                        mybir.ActivationFunctionType.Exp,
                        scale=0.125,
                    )

                    # ---- xB = x @ B per batch ----
                    for h in (0, 1):
                        for bl in range(4):
                            nc.tensor.matmul(
                                wb_ps[h * C : (h + 1) * C, 1, bl, :],
                                xT[h * C : (h + 1) * C, bl * T : (bl + 1) * T],
                                B2[h * C : (h + 1) * C, :],
                                tile_position=(h * C, h * C),
                            )
                    xB = work_pool.tile([128, 4, C + 1], BF16, tag="xB")
                    nc.vector.tensor_tensor(
                        xB[:, :, 0:C],
                        wb_ps[:, 1],
                        bias_bc[:].unsqueeze(1).broadcast_to([128, 4, C]),
                        ADD,
                    )
                    nc.vector.memset(xB[:, :, C : C + 1], 1.0)

                    # ---- y | sums: even/odd token split -> 2 tok/partition ----
                    # two full psum banks: h selects the bank (max 8 matmul
                    # groups per bank); slots 2..3 of dim 'a' are padding
                    y_ps = ps.tile([128, 2, 4, 2 * C], F32, tag="y")
                    for h in (0, 1):
                        for bl in range(4):
                            for par in (0, 1):
                                nc.tensor.matmul(
                                    y_ps[bl * 32 : (bl + 1) * 32, h, par, 0 : C + 1],
                                    weiT_e[h * C : (h + 1) * C, bl, par::2],
                                    xB[h * C : (h + 1) * C, bl, :],
                                    tile_position=(h * C, bl * 32),
                                )
                    recip = work_pool.tile([128, 2, 2], F32, tag="recip")
                    nc.vector.reciprocal(recip[:], y_ps[:, :, 0:2, C : C + 1])
                    nc.vector.tensor_tensor(
                        y4[:, u, :, :, :],
                        y_ps[:, :, 0:2, 0:C],
                        recip[:].unsqueeze(3).broadcast_to([128, 2, 2, C]),
                        MULT,
                    )
                # out DMA on ACT hwdge queue; 512B descriptors
                nc.sync.dma_start(
                    out_flat[base : base + TPC * TOK, :].rearrange(
                        "(u j p m) c -> p u j (m c)", u=TPC, j=2, p=128, m=2
                    ),
                    y4[:].rearrange("p u j m c -> p u j (m c)"),
                )
            if rep_ctx is not None:
                rep_ctx.__exit__(None, None, None)

    nc.compile()
    return nc


_NC_CACHE = {}


def _get_nc(b_loc, batches_per_tile=8):
    key = (b_loc, batches_per_tile)
    if key not in _NC_CACHE:
        _NC_CACHE[key] = build_nc(b_loc, batches_per_tile)
    return _NC_CACHE[key]


def kernel(x, Wk, Wq, Wv, Wp, bp):
    from concourse.bass_utils import run_bass_kernel_spmd

    x = np.ascontiguousarray(x, dtype=np.float32)
    weights = {
        "Wk": np.ascontiguousarray(Wk, dtype=np.float32),
        "Wq": np.ascontiguousarray(Wq, dtype=np.float32),
        "Wv": np.ascontiguousarray(Wv, dtype=np.float32),
        "Wp": np.ascontiguousarray(Wp, dtype=np.float32),
        "bp": np.ascontiguousarray(bp, dtype=np.float32),
    }
    nc = _get_nc(B_LOC)
    in_maps = [
        {"x": x[i * B_LOC : (i + 1) * B_LOC], **weights} for i in range(N_CORES)
    ]
    res = run_bass_kernel_spmd(nc, in_maps, core_ids=list(range(N_CORES)))
    outs = [res.results[i]["out"] for i in range(N_CORES)]
    return np.concatenate(outs, axis=0)
